# revision 40
# baseline (speedup 1.0000x reference)
"""MoE layer (8 experts, top-2) on 8 TRN2 NeuronCores, expert-parallel.

Host does the router + dispatch/combine; each core runs the two FFN matmuls
for one expert on its gathered tokens in bf16 (fp32 PSUM accumulation).

Two serial phases, both with static weight tiles as the matmul stationary
operand so the PE weight loads hide under the previous matmul's stream
(stationary tiles freshly written by another engine stall the load on the
producer semaphore — measured +40ns/matmul on the fp32r baseline):
  A: h[i-slab, tok] = relu(W1-block.T @ x-chunk + b1), 512 matmuls
  B: yT[h-block, tok] = W2-block.T @ h-chunk,          512 matmuls
Output is the transposed yT = (relu(x@W1+b1) @ W2).T; the host transposes,
scales by the router weight and adds w*b2 during the combine.

Self-contained: hardcodes shapes HIDDEN=1024, INNER=2048, NUM_EXPERTS=8,
TOP_K=2.
"""

import sys

import numpy as np
import ml_dtypes

try:
    import concourse.bass as bass  # noqa: F401
except ImportError:
    sys.path.insert(0, "/opt/trn_rl_repo")

import concourse.tile as tile
from concourse import bacc, mybir
from concourse.bass_utils import run_bass_kernel_spmd

H = 1024
INNER = 2048
E = 8
TOP_K = 2
N_D = H // 128  # 8 k-tiles for matmul A
N_I = INNER // 128  # 16 k-tiles for matmul B
N_H = H // 128  # 8 output row-blocks
TCH = 512  # token chunk (moving free dim, PE max)

F32 = mybir.dt.float32
BF16 = mybir.dt.bfloat16
NPBF16 = ml_dtypes.bfloat16
RELU = mybir.ActivationFunctionType.Relu

# test.py hooks: set TRACE=True before calling kernel() to profile;
# LAST_RESULT then holds the BassKernelResults (exec_time_ns etc.).
TRACE = False
TRACE_KWARGS = {}
LAST_RESULT = None

_cache = {}


def _unpack_y(yt, c):
    # yt [N_H, ci, 128, TCH] -> yT [H, c] float32
    yt = np.asarray(yt, dtype=np.float32)
    return yt.transpose(0, 2, 1, 3).reshape(H, c)


def _build(c):
    n_ch = c // TCH
    nc = bacc.Bacc("TRN2", target_bir_lowering=False, debug=False, num_devices=8)

    # All DRAM tensors are packed on the host so every DMA moves a
    # contiguous region — the DMA engine fragments strided transfers into
    # per-line descriptors at ~47ns/line, which throttles 1KB-line tiles
    # to ~21GB/s/queue and turns the output drain into a ~10us tail.
    # xt[ci][d] = xT[d*128:(d+1)*128, ci*512:(ci+1)*512]
    xt = nc.dram_tensor("xt", [n_ch, N_D, 128, TCH], BF16, kind="ExternalInput")
    # W1 pre-tiled on host into inner-dim slabs: w1t[i][p, d*128+m] =
    # W1[d*128+p, i*128+m], so slab i holds all stationary blocks for
    # phase-A step i. Column H carries the slab's bias b1[i*128+p] (bf16
    # bias costs ~1e-4 absolute — noise) so no separate b1 DMA: even a
    # 64B/partition transfer walks one ~47ns fragment per partition, i.e.
    # ~6us of a DMA walker slot in the critical warmup window.
    w1 = nc.dram_tensor("w1t", [N_I, 128, H + 1], BF16, kind="ExternalInput")
    w2 = nc.dram_tensor("w2", [INNER, H], BF16, kind="ExternalInput")
    # yt[hi][ci] = yT[hi*128:(hi+1)*128, ci*512:(ci+1)*512]
    yt = nc.dram_tensor("yt", [N_H, n_ch, 128, TCH], BF16, kind="ExternalOutput")

    with tile.TileContext(nc, pool_alloc_mode="queue") as tc:
        with (
            tc.tile_pool(name="weights", bufs=1) as wpool,
            tc.tile_pool(name="tokens", bufs=1) as tpool,
            tc.tile_pool(name="hidden", bufs=1) as hpool,
            tc.tile_pool(name="out", bufs=4) as opool,
            tc.tile_pool(name="psum", bufs=8, space="PSUM") as ps,
        ):
            w1_sb = [
                wpool.tile([128, H + 1], BF16, tag=f"w1_{i}", name=f"w1s_{i}")
                for i in range(N_I)
            ]
            w2_sb = [
                wpool.tile([128, H], BF16, tag=f"w2_{i}", name=f"w2s_{i}")
                for i in range(N_I)
            ]
            # token chunk c as two tiles (d=0..3 / d=4..7) so the first
            # matmul only gates on half the chunk's DMA
            tt = [
                (
                    tpool.tile([128, 4 * TCH], BF16, tag=f"tl_{ci}", name=f"tl_{ci}"),
                    tpool.tile([128, 4 * TCH], BF16, tag=f"th_{ci}", name=f"th_{ci}"),
                )
                for ci in range(n_ch)
            ]
            hh = [
                hpool.tile([128, N_I * TCH], BF16, tag=f"h_{ci}", name=f"h_{ci}")
                for ci in range(n_ch)
            ]

            def tok_slice(ci, d):
                lo, hi = tt[ci]
                t = lo if d < 4 else hi
                return t[:, (d % 4) * TCH:(d % 4 + 1) * TCH]

            # DMA order = consumption order. Descriptor issue costs ~0.3us
            # each on the sync sequencer, so keep the count low: one
            # contiguous 128-256KB burst per tile region. Phase A consumes
            # a w1 slab every ~1.7us; later token chunks (needed at
            # 27/55/82us) and w2 (needed at B start, ~110us) go last.
            nc.sync.dma_start(w1_sb[0][:], w1.ap()[0])
            for d in range(N_D):
                nc.sync.dma_start(tok_slice(0, d), xt.ap()[0, d])
            nc.sync.dma_start(w1_sb[1][:], w1.ap()[1])
            nc.sync.dma_start(w1_sb[2][:], w1.ap()[2])
            for i in range(3, N_I):
                nc.sync.dma_start(w1_sb[i][:], w1.ap()[i])
            if n_ch > 1:
                for d in range(N_D):
                    nc.sync.dma_start(tok_slice(1, d), xt.ap()[1, d])
            for i in range(N_I):
                nc.sync.dma_start(w2_sb[i][:], w2.ap()[i * 128:(i + 1) * 128, :])
            for ci in range(2, n_ch):
                for d in range(N_D):
                    nc.sync.dma_start(tok_slice(ci, d), xt.ap()[ci, d])

            # Phase A: h = relu(x @ W1 + b1), h laid out [inner-part, tok]
            for ci in range(n_ch):
                for i in range(N_I):
                    pa = ps.tile([128, TCH], F32, tag="p")
                    for d in range(N_D):
                        nc.tensor.matmul(
                            pa[:],
                            w1_sb[i][:, d * 128:(d + 1) * 128],
                            tok_slice(ci, d),
                            start=(d == 0),
                            stop=(d == N_D - 1),
                        )
                    nc.scalar.activation(
                        hh[ci][:, i * TCH:(i + 1) * TCH],
                        pa[:],
                        RELU,
                        bias=w1_sb[i][:, H:H + 1],
                    )

            # Phase B: yT = (h.T @ W2).T, stationary = W2 blocks
            for hi in range(N_H):
                for ci in range(n_ch):
                    pb = ps.tile([128, TCH], F32, tag="p")
                    for i in range(N_I):
                        nc.tensor.matmul(
                            pb[:],
                            w2_sb[i][:, hi * 128:(hi + 1) * 128],
                            hh[ci][:, i * TCH:(i + 1) * TCH],
                            start=(i == 0),
                            stop=(i == N_I - 1),
                        )
                    oo = opool.tile([128, TCH], BF16, tag="o")
                    nc.scalar.copy(oo[:], pb[:])
                    if hi == N_H - 1:
                        # final drain: a transfer walks ~47ns per partition
                        # regardless of width, so shrink the fragment count
                        # of the last group's stores (pieces walk in
                        # parallel)
                        pieces = 4 if ci == n_ch - 1 else 2
                        step = 128 // pieces
                        for h in range(pieces):
                            nc.sync.dma_start(
                                yt.ap()[hi, ci, h * step:(h + 1) * step, :],
                                oo[h * step:(h + 1) * step, :],
                            )
                    else:
                        nc.sync.dma_start(yt.ap()[hi, ci], oo[:])

    nc.compile()
    return nc


def kernel(x, Wr, br, W1, b1, W2, b2):
    global LAST_RESULT
    x = np.asarray(x, dtype=np.float32)
    Wr = np.asarray(Wr, dtype=np.float32)
    br = np.asarray(br, dtype=np.float32)
    W1 = np.asarray(W1, dtype=np.float32)
    b1 = np.asarray(b1, dtype=np.float32)
    W2 = np.asarray(W2, dtype=np.float32)
    b2 = np.asarray(b2, dtype=np.float32)

    batch, seq, hidden = x.shape
    x2d = x.reshape(-1, hidden)
    n = x2d.shape[0]

    # Router (matches jax reference: top-2 descending, stable ties, softmax).
    logits = x2d @ Wr + br
    order = np.argsort(-logits, axis=1, kind="stable")[:, :TOP_K]
    l0 = logits[np.arange(n), order[:, 0]]
    l1 = logits[np.arange(n), order[:, 1]]
    e1 = np.exp(l1 - l0)
    denom = 1.0 + e1
    top_w = np.stack([1.0 / denom, e1 / denom], axis=1).astype(np.float32)

    rows_l, wsel_l = [], []
    for e in range(E):
        rows, cols = np.nonzero(order == e)
        rows_l.append(rows)
        wsel_l.append(top_w[rows, cols])
    counts = np.array([len(r) for r in rows_l])

    # Expert capacity: perfect-balance point (n*TOP_K/E). Overflow tokens
    # of hot experts are computed on the host in fp32 during the combine.
    cap = n * TOP_K // E
    c = max(TCH, min(int(-(-counts.max() // TCH)) * TCH, cap))

    if c not in _cache:
        _cache[c] = _build(c)
    nc = _cache[c]

    in_maps = []
    pad_ref = []
    for e in range(E):
        rows = rows_l[e][:c]
        ne = len(rows)
        xTe = np.zeros((H, c), dtype=NPBF16)
        xTe[:, :ne] = x2d[rows].T.astype(NPBF16)
        # pack to [ci, d, 128, TCH] so each DMA is a contiguous burst
        xte = np.ascontiguousarray(
            xTe.reshape(N_D, 128, c // TCH, TCH).transpose(2, 0, 1, 3)
        )
        w1t = np.empty((N_I, 128, H + 1), dtype=NPBF16)
        w1t[:, :, :H] = (
            W1[e].reshape(N_D, 128, N_I, 128).transpose(2, 1, 0, 3).reshape(N_I, 128, H)
        ).astype(NPBF16)
        w1t[:, :, H] = b1[e].reshape(N_I, 128).astype(NPBF16)
        in_maps.append(
            {
                "xt": xte,
                "w1t": w1t,
                "w2": W2[e].astype(NPBF16),
            }
        )
        # padded token columns all compute yT_pad = (relu(b1) @ W2).T
        pad_ref.append(np.maximum(b1[e], 0.0) @ W2[e])

    # Host fp32 reference for a few sampled real tokens per expert: the
    # device occasionally returns subtly corrupted data (~2e-2-level errors)
    # in the used region that the pad-column canary cannot see.
    spot_cols, spot_ref = [], []
    for e in range(E):
        ne = len(rows_l[e][:c])
        cols = (
            np.unique(np.linspace(0, ne - 1, 16).astype(int))
            if ne > 0
            else np.zeros(0, dtype=int)
        )
        xs = x2d[rows_l[e][cols]]
        hs = np.maximum(xs @ W1[e] + b1[e], 0.0)
        spot_cols.append(cols)
        spot_ref.append((hs @ W2[e]).T)  # [H, len(cols)]

    # The device occasionally drops a run (NRT_EXEC_UNIT_UNRECOVERABLE) and
    # the run after a drop can return garbage. Padded columns must come back
    # (a) bit-identical to each other and (b) close to the host-computed
    # relu(b1)@W2 — use that as an integrity canary and retry on failure.
    res = None
    for attempt in range(4):
        try:
            res = run_bass_kernel_spmd(
                nc, in_maps, list(range(E)), trace=TRACE, **TRACE_KWARGS
            )
        except Exception:
            if attempt == 3:
                raise
            continue
        ok = True
        for e in range(E):
            ye = _unpack_y(res.results[e]["yt"], c)
            ne = len(rows_l[e][:c])
            if not np.isfinite(ye).all():
                ok = False
                break
            if (
                spot_cols[e].size
                and np.abs(ye[:, spot_cols[e]] - spot_ref[e]).max() > 1e-2
            ):
                ok = False
                break
            if ne < c:
                v = pad_ref[e]
                tol = 0.05 * max(np.abs(v).max(), 1e-2)
                if (
                    np.abs(ye[:, ne:] - v[:, None]).max() > tol
                    or not (ye[:, ne:] == ye[:, -1:]).all()
                ):
                    ok = False
                    break
        if ok:
            break
    LAST_RESULT = res

    out = np.zeros((n, hidden), dtype=np.float32)
    for e in range(E):
        rows = rows_l[e][:c]
        ne = len(rows)
        w = wsel_l[e][:ne, None]
        ye = _unpack_y(res.results[e]["yt"], c)
        out[rows] += w * ye[:, :ne].T + w * b2[e][None, :]
        if len(rows_l[e]) > c:  # overflow tokens: full-precision host FFN
            rov = rows_l[e][c:]
            wov = wsel_l[e][c:, None]
            hov = np.maximum(x2d[rov] @ W1[e] + b1[e], 0.0)
            out[rov] += wov * (hov @ W2[e] + b2[e])
    return out.reshape(batch, seq, hidden)


# revision 44
# speedup vs baseline: 1.0031x; 1.0031x over previous
"""MoE layer (8 experts, top-2) on 8 TRN2 NeuronCores, expert-parallel.

Host does the router + dispatch/combine; each core runs the two FFN matmuls
for one expert on its gathered tokens in bf16 (fp32 PSUM accumulation).

Two serial phases, both with static weight tiles as the matmul stationary
operand so the PE weight loads hide under the previous matmul's stream
(stationary tiles freshly written by another engine stall the load on the
producer semaphore — measured +40ns/matmul on the fp32r baseline):
  A: h[i-slab, tok] = relu(W1-block.T @ x-chunk + b1), 512 matmuls
  B: yT[h-block, tok] = W2-block.T @ h-chunk,          512 matmuls
Output is the transposed yT = (relu(x@W1+b1) @ W2).T; the host transposes,
scales by the router weight and adds w*b2 during the combine.

Self-contained: hardcodes shapes HIDDEN=1024, INNER=2048, NUM_EXPERTS=8,
TOP_K=2.
"""

import sys

import numpy as np
import ml_dtypes

try:
    import concourse.bass as bass  # noqa: F401
except ImportError:
    sys.path.insert(0, "/opt/trn_rl_repo")

import concourse.tile as tile
from concourse import bacc, mybir
from concourse.bass_utils import run_bass_kernel_spmd

H = 1024
INNER = 2048
E = 8
TOP_K = 2
N_D = H // 128  # 8 k-tiles for matmul A
N_I = INNER // 128  # 16 k-tiles for matmul B
N_H = H // 128  # 8 output row-blocks
TCH = 512  # token chunk (moving free dim, PE max)

F32 = mybir.dt.float32
BF16 = mybir.dt.bfloat16
NPBF16 = ml_dtypes.bfloat16
RELU = mybir.ActivationFunctionType.Relu

# test.py hooks: set TRACE=True before calling kernel() to profile;
# LAST_RESULT then holds the BassKernelResults (exec_time_ns etc.).
TRACE = False
TRACE_KWARGS = {}
LAST_RESULT = None

_cache = {}


def _unpack_y(yt, c):
    # yt [N_H, ci, 128, TCH] -> yT [H, c] float32
    yt = np.asarray(yt, dtype=np.float32)
    return yt.transpose(0, 2, 1, 3).reshape(H, c)


def _build(c):
    n_ch = c // TCH
    nc = bacc.Bacc("TRN2", target_bir_lowering=False, debug=False, num_devices=8)

    # All DRAM tensors are packed on the host so every DMA moves a
    # contiguous region — the DMA engine fragments strided transfers into
    # per-line descriptors at ~47ns/line, which throttles 1KB-line tiles
    # to ~21GB/s/queue and turns the output drain into a ~10us tail.
    # xt[ci][d] = xT[d*128:(d+1)*128, ci*512:(ci+1)*512]
    xt = nc.dram_tensor("xt", [n_ch, N_D, 128, TCH], BF16, kind="ExternalInput")
    # W1 pre-tiled on host into inner-dim slabs: w1t[i][p, d*128+m] =
    # W1[d*128+p, i*128+m], so slab i holds all stationary blocks for
    # phase-A step i. Column H carries the slab's bias b1[i*128+p] (bf16
    # bias costs ~1e-4 absolute — noise) so no separate b1 DMA: even a
    # 64B/partition transfer walks one ~47ns fragment per partition, i.e.
    # ~6us of a DMA walker slot in the critical warmup window.
    w1 = nc.dram_tensor("w1t", [N_I, 128, H + 1], BF16, kind="ExternalInput")
    w2 = nc.dram_tensor("w2", [INNER, H], BF16, kind="ExternalInput")
    # yt[hi][ci] = yT[hi*128:(hi+1)*128, ci*512:(ci+1)*512]
    yt = nc.dram_tensor("yt", [N_H, n_ch, 128, TCH], BF16, kind="ExternalOutput")

    with tile.TileContext(nc, pool_alloc_mode="queue") as tc:
        with (
            tc.tile_pool(name="weights", bufs=1) as wpool,
            tc.tile_pool(name="tokens", bufs=1) as tpool,
            tc.tile_pool(name="hidden", bufs=1) as hpool,
            tc.tile_pool(name="out", bufs=4) as opool,
            tc.tile_pool(name="psum", bufs=8, space="PSUM") as ps,
        ):
            warm = wpool.tile([128, 640], BF16, tag="warm")
            w1_sb = [
                wpool.tile([128, H + 1], BF16, tag=f"w1_{i}", name=f"w1s_{i}")
                for i in range(N_I)
            ]
            w2_sb = [
                wpool.tile([128, H], BF16, tag=f"w2_{i}", name=f"w2s_{i}")
                for i in range(N_I)
            ]
            # token chunk c as two tiles (d=0..3 / d=4..7) so the first
            # matmul only gates on half the chunk's DMA
            tt = [
                (
                    tpool.tile([128, 4 * TCH], BF16, tag=f"tl_{ci}", name=f"tl_{ci}"),
                    tpool.tile([128, 4 * TCH], BF16, tag=f"th_{ci}", name=f"th_{ci}"),
                )
                for ci in range(n_ch)
            ]
            hh = [
                hpool.tile([128, N_I * TCH], BF16, tag=f"h_{ci}", name=f"h_{ci}")
                for ci in range(n_ch)
            ]

            def tok_slice(ci, d):
                lo, hi = tt[ci]
                t = lo if d < 4 else hi
                return t[:, (d % 4) * TCH:(d % 4 + 1) * TCH]

            # PE pre-warm: the pstate governor needs ~3us of continuous
            # execution before the PE runs at 2.4GHz (cold matmuls take
            # 427-609ns instead of 216ns). Bridge the initial DMA wait with
            # dummy matmuls on a memset tile (vector engine, no DMA dep),
            # aimed at the first real psum tile — its real group opens with
            # start=True, which resets the bank, so the junk is discarded
            # and the tile keeps its reader.
            nc.vector.memset(warm[:], 1.0)
            pa0 = ps.tile([128, TCH], F32, tag="p")
            for _ in range(10):
                nc.tensor.matmul(
                    pa0[:], warm[:, 0:128], warm[:, 128:640], start=True, stop=True
                )

            # DMA order = strict consumption order. Descriptor issue costs
            # ~0.3us each on the sync sequencer, so keep the count low: one
            # contiguous 128-256KB burst per tile region. Phase A consumes
            # a w1 slab every ~1.7us; token chunks are needed at 27/55/82us
            # and w2 only at B start (~118us), so w2 goes dead last to keep
            # it out of the contended warmup window.
            nc.sync.dma_start(w1_sb[0][:], w1.ap()[0])
            for d in range(N_D):
                nc.sync.dma_start(tok_slice(0, d), xt.ap()[0, d])
            nc.sync.dma_start(w1_sb[1][:], w1.ap()[1])
            nc.sync.dma_start(w1_sb[2][:], w1.ap()[2])
            for i in range(3, N_I):
                nc.sync.dma_start(w1_sb[i][:], w1.ap()[i])
            for ci in range(1, n_ch):
                for d in range(N_D):
                    nc.sync.dma_start(tok_slice(ci, d), xt.ap()[ci, d])
            for i in range(N_I):
                nc.sync.dma_start(w2_sb[i][:], w2.ap()[i * 128:(i + 1) * 128, :])

            # Phase A: h = relu(x @ W1 + b1), h laid out [inner-part, tok]
            for ci in range(n_ch):
                for i in range(N_I):
                    pa = pa0 if (ci == 0 and i == 0) else ps.tile([128, TCH], F32, tag="p")
                    for d in range(N_D):
                        nc.tensor.matmul(
                            pa[:],
                            w1_sb[i][:, d * 128:(d + 1) * 128],
                            tok_slice(ci, d),
                            start=(d == 0),
                            stop=(d == N_D - 1),
                        )
                    nc.scalar.activation(
                        hh[ci][:, i * TCH:(i + 1) * TCH],
                        pa[:],
                        RELU,
                        bias=w1_sb[i][:, H:H + 1],
                    )

            # Phase B: yT = (h.T @ W2).T, stationary = W2 blocks
            for hi in range(N_H):
                for ci in range(n_ch):
                    pb = ps.tile([128, TCH], F32, tag="p")
                    for i in range(N_I):
                        nc.tensor.matmul(
                            pb[:],
                            w2_sb[i][:, hi * 128:(hi + 1) * 128],
                            hh[ci][:, i * TCH:(i + 1) * TCH],
                            start=(i == 0),
                            stop=(i == N_I - 1),
                        )
                    oo = opool.tile([128, TCH], BF16, tag="o")
                    nc.scalar.copy(oo[:], pb[:])
                    if hi == N_H - 1:
                        # final drain: a transfer walks ~47ns per partition
                        # regardless of width, so shrink the fragment count
                        # of the last group's stores (pieces walk in
                        # parallel)
                        pieces = 4 if ci == n_ch - 1 else 2
                        step = 128 // pieces
                        for h in range(pieces):
                            nc.sync.dma_start(
                                yt.ap()[hi, ci, h * step:(h + 1) * step, :],
                                oo[h * step:(h + 1) * step, :],
                            )
                    else:
                        nc.sync.dma_start(yt.ap()[hi, ci], oo[:])

    nc.compile()
    return nc


def kernel(x, Wr, br, W1, b1, W2, b2):
    global LAST_RESULT
    x = np.asarray(x, dtype=np.float32)
    Wr = np.asarray(Wr, dtype=np.float32)
    br = np.asarray(br, dtype=np.float32)
    W1 = np.asarray(W1, dtype=np.float32)
    b1 = np.asarray(b1, dtype=np.float32)
    W2 = np.asarray(W2, dtype=np.float32)
    b2 = np.asarray(b2, dtype=np.float32)

    batch, seq, hidden = x.shape
    x2d = x.reshape(-1, hidden)
    n = x2d.shape[0]

    # Router (matches jax reference: top-2 descending, stable ties, softmax).
    logits = x2d @ Wr + br
    order = np.argsort(-logits, axis=1, kind="stable")[:, :TOP_K]
    l0 = logits[np.arange(n), order[:, 0]]
    l1 = logits[np.arange(n), order[:, 1]]
    e1 = np.exp(l1 - l0)
    denom = 1.0 + e1
    top_w = np.stack([1.0 / denom, e1 / denom], axis=1).astype(np.float32)

    rows_l, wsel_l = [], []
    for e in range(E):
        rows, cols = np.nonzero(order == e)
        rows_l.append(rows)
        wsel_l.append(top_w[rows, cols])
    counts = np.array([len(r) for r in rows_l])

    # Expert capacity: perfect-balance point (n*TOP_K/E). Overflow tokens
    # of hot experts are computed on the host in fp32 during the combine.
    cap = n * TOP_K // E
    c = max(TCH, min(int(-(-counts.max() // TCH)) * TCH, cap))

    if c not in _cache:
        _cache[c] = _build(c)
    nc = _cache[c]

    in_maps = []
    pad_ref = []
    for e in range(E):
        rows = rows_l[e][:c]
        ne = len(rows)
        xTe = np.zeros((H, c), dtype=NPBF16)
        xTe[:, :ne] = x2d[rows].T.astype(NPBF16)
        # pack to [ci, d, 128, TCH] so each DMA is a contiguous burst
        xte = np.ascontiguousarray(
            xTe.reshape(N_D, 128, c // TCH, TCH).transpose(2, 0, 1, 3)
        )
        w1t = np.empty((N_I, 128, H + 1), dtype=NPBF16)
        w1t[:, :, :H] = (
            W1[e].reshape(N_D, 128, N_I, 128).transpose(2, 1, 0, 3).reshape(N_I, 128, H)
        ).astype(NPBF16)
        w1t[:, :, H] = b1[e].reshape(N_I, 128).astype(NPBF16)
        in_maps.append(
            {
                "xt": xte,
                "w1t": w1t,
                "w2": W2[e].astype(NPBF16),
            }
        )
        # padded token columns all compute yT_pad = (relu(b1) @ W2).T
        pad_ref.append(np.maximum(b1[e], 0.0) @ W2[e])

    # Host fp32 reference for a few sampled real tokens per expert: the
    # device occasionally returns subtly corrupted data (~2e-2-level errors)
    # in the used region that the pad-column canary cannot see.
    spot_cols, spot_ref = [], []
    for e in range(E):
        ne = len(rows_l[e][:c])
        cols = (
            np.unique(np.linspace(0, ne - 1, 16).astype(int))
            if ne > 0
            else np.zeros(0, dtype=int)
        )
        xs = x2d[rows_l[e][cols]]
        hs = np.maximum(xs @ W1[e] + b1[e], 0.0)
        spot_cols.append(cols)
        spot_ref.append((hs @ W2[e]).T)  # [H, len(cols)]

    # The device occasionally drops a run (NRT_EXEC_UNIT_UNRECOVERABLE) and
    # the run after a drop can return garbage. Padded columns must come back
    # (a) bit-identical to each other and (b) close to the host-computed
    # relu(b1)@W2 — use that as an integrity canary and retry on failure.
    res = None
    for attempt in range(4):
        try:
            res = run_bass_kernel_spmd(
                nc, in_maps, list(range(E)), trace=TRACE, **TRACE_KWARGS
            )
        except Exception:
            if attempt == 3:
                raise
            continue
        ok = True
        for e in range(E):
            ye = _unpack_y(res.results[e]["yt"], c)
            ne = len(rows_l[e][:c])
            if not np.isfinite(ye).all():
                ok = False
                break
            if (
                spot_cols[e].size
                and np.abs(ye[:, spot_cols[e]] - spot_ref[e]).max() > 1e-2
            ):
                ok = False
                break
            if ne < c:
                v = pad_ref[e]
                tol = 0.05 * max(np.abs(v).max(), 1e-2)
                if (
                    np.abs(ye[:, ne:] - v[:, None]).max() > tol
                    or not (ye[:, ne:] == ye[:, -1:]).all()
                ):
                    ok = False
                    break
        if ok:
            break
    LAST_RESULT = res

    out = np.zeros((n, hidden), dtype=np.float32)
    for e in range(E):
        rows = rows_l[e][:c]
        ne = len(rows)
        w = wsel_l[e][:ne, None]
        ye = _unpack_y(res.results[e]["yt"], c)
        out[rows] += w * ye[:, :ne].T + w * b2[e][None, :]
        if len(rows_l[e]) > c:  # overflow tokens: full-precision host FFN
            rov = rows_l[e][c:]
            wov = wsel_l[e][c:, None]
            hov = np.maximum(x2d[rov] @ W1[e] + b1[e], 0.0)
            out[rov] += wov * (hov @ W2[e] + b2[e])
    return out.reshape(batch, seq, hidden)


# revision 47
# speedup vs baseline: 1.0110x; 1.0079x over previous
"""MoE layer (8 experts, top-2) on 8 TRN2 NeuronCores, expert-parallel.

Host does the router + dispatch/combine; each core runs the two FFN matmuls
for one expert on its gathered tokens in bf16 (fp32 PSUM accumulation).

Two serial phases, both with static weight tiles as the matmul stationary
operand so the PE weight loads hide under the previous matmul's stream
(stationary tiles freshly written by another engine stall the load on the
producer semaphore — measured +40ns/matmul on the fp32r baseline):
  A: h[i-slab, tok] = relu(W1-block.T @ x-chunk + b1), 512 matmuls
  B: yT[h-block, tok] = W2-block.T @ h-chunk,          512 matmuls
Output is the transposed yT = (relu(x@W1+b1) @ W2).T; the host transposes,
scales by the router weight and adds w*b2 during the combine.

Self-contained: hardcodes shapes HIDDEN=1024, INNER=2048, NUM_EXPERTS=8,
TOP_K=2.
"""

import sys

import numpy as np
import ml_dtypes

try:
    import concourse.bass as bass  # noqa: F401
except ImportError:
    sys.path.insert(0, "/opt/trn_rl_repo")

import concourse.tile as tile
from concourse import bacc, mybir
from concourse.bass_utils import run_bass_kernel_spmd

H = 1024
INNER = 2048
E = 8
TOP_K = 2
N_D = H // 128  # 8 k-tiles for matmul A
N_I = INNER // 128  # 16 k-tiles for matmul B
N_H = H // 128  # 8 output row-blocks
TCH = 512  # token chunk (moving free dim, PE max)

F32 = mybir.dt.float32
BF16 = mybir.dt.bfloat16
NPBF16 = ml_dtypes.bfloat16
RELU = mybir.ActivationFunctionType.Relu

# test.py hooks: set TRACE=True before calling kernel() to profile;
# LAST_RESULT then holds the BassKernelResults (exec_time_ns etc.).
TRACE = False
TRACE_KWARGS = {}
LAST_RESULT = None

_cache = {}


def _unpack_y(yt, c):
    # yt [N_H, 128, c] -> yT [H, c] float32
    return np.asarray(yt, dtype=np.float32).reshape(H, c)


def _build(c):
    n_ch = c // TCH
    nc = bacc.Bacc("TRN2", target_bir_lowering=False, debug=False, num_devices=8)

    # All DRAM tensors are packed on the host so every DMA moves a
    # contiguous region — the DMA engine fragments strided transfers into
    # per-line descriptors at ~47ns/line, which throttles 1KB-line tiles
    # to ~21GB/s/queue and turns the output drain into a ~10us tail.
    # xt[ci][d] = xT[d*128:(d+1)*128, ci*512:(ci+1)*512]
    xt = nc.dram_tensor("xt", [n_ch, N_D, 128, TCH], BF16, kind="ExternalInput")
    # W1 pre-tiled on host into inner-dim slabs: w1t[i][p, d*128+m] =
    # W1[d*128+p, i*128+m], so slab i holds all stationary blocks for
    # phase-A step i. Column H carries the slab's bias b1[i*128+p] (bf16
    # bias costs ~1e-4 absolute — noise) so no separate b1 DMA: even a
    # 64B/partition transfer walks one ~47ns fragment per partition, i.e.
    # ~6us of a DMA walker slot in the critical warmup window.
    w1 = nc.dram_tensor("w1t", [N_I, 128, H + 1], BF16, kind="ExternalInput")
    w2 = nc.dram_tensor("w2", [INNER, H], BF16, kind="ExternalInput")
    # yt[hi][p][tok] = yT[hi*128+p, tok] — i.e. plain yT [H, c] rows.
    # Stores go out as [128, 2*TCH] chunk-pairs: 2KB contiguous per
    # partition row, so each walk moves twice the bytes of a single-chunk
    # store for the same ~47ns/partition fragment cost.
    yt = nc.dram_tensor("yt", [N_H, 128, c], BF16, kind="ExternalOutput")

    with tile.TileContext(nc, pool_alloc_mode="queue") as tc:
        with (
            tc.tile_pool(name="weights", bufs=1) as wpool,
            tc.tile_pool(name="tokens", bufs=1) as tpool,
            tc.tile_pool(name="hidden", bufs=1) as hpool,
            tc.tile_pool(name="out", bufs=4) as opool,
            tc.tile_pool(name="psum", bufs=8, space="PSUM") as ps,
        ):
            warm = wpool.tile([128, 640], BF16, tag="warm")
            w1_sb = [
                wpool.tile([128, H + 1], BF16, tag=f"w1_{i}", name=f"w1s_{i}")
                for i in range(N_I)
            ]
            w2_sb = [
                wpool.tile([128, H], BF16, tag=f"w2_{i}", name=f"w2s_{i}")
                for i in range(N_I)
            ]
            # token chunk c as two tiles (d=0..3 / d=4..7) so the first
            # matmul only gates on half the chunk's DMA
            tt = [
                (
                    tpool.tile([128, 4 * TCH], BF16, tag=f"tl_{ci}", name=f"tl_{ci}"),
                    tpool.tile([128, 4 * TCH], BF16, tag=f"th_{ci}", name=f"th_{ci}"),
                )
                for ci in range(n_ch)
            ]
            hh = [
                hpool.tile([128, N_I * TCH], BF16, tag=f"h_{ci}", name=f"h_{ci}")
                for ci in range(n_ch)
            ]

            def tok_slice(ci, d):
                lo, hi = tt[ci]
                t = lo if d < 4 else hi
                return t[:, (d % 4) * TCH:(d % 4 + 1) * TCH]

            # PE pre-warm: the pstate governor needs ~3us of continuous
            # execution before the PE runs at 2.4GHz (cold matmuls take
            # 427-609ns instead of 216ns). Bridge the initial DMA wait with
            # dummy matmuls on a memset tile (vector engine, no DMA dep),
            # aimed at the first real psum tile — its real group opens with
            # start=True, which resets the bank, so the junk is discarded
            # and the tile keeps its reader.
            nc.vector.memset(warm[:], 1.0)
            pa0 = ps.tile([128, TCH], F32, tag="p")
            for _ in range(10):
                nc.tensor.matmul(
                    pa0[:], warm[:, 0:128], warm[:, 128:640], start=True, stop=True
                )

            # DMA order = strict consumption order. Descriptor issue costs
            # ~0.3us each on the sync sequencer, so keep the count low: one
            # contiguous 128-256KB burst per tile region. Phase A consumes
            # a w1 slab every ~1.7us; token chunks are needed at 27/55/82us
            # and w2 only at B start (~118us), so w2 goes dead last to keep
            # it out of the contended warmup window.
            nc.sync.dma_start(w1_sb[0][:], w1.ap()[0])
            for d in range(N_D):
                nc.sync.dma_start(tok_slice(0, d), xt.ap()[0, d])
            nc.sync.dma_start(w1_sb[1][:], w1.ap()[1])
            nc.sync.dma_start(w1_sb[2][:], w1.ap()[2])
            for i in range(3, N_I):
                nc.sync.dma_start(w1_sb[i][:], w1.ap()[i])
            for ci in range(1, n_ch):
                for d in range(N_D):
                    nc.sync.dma_start(tok_slice(ci, d), xt.ap()[ci, d])
            for i in range(N_I):
                nc.sync.dma_start(w2_sb[i][:], w2.ap()[i * 128:(i + 1) * 128, :])

            # Phase A: h = relu(x @ W1 + b1), h laid out [inner-part, tok]
            for ci in range(n_ch):
                for i in range(N_I):
                    pa = pa0 if (ci == 0 and i == 0) else ps.tile([128, TCH], F32, tag="p")
                    for d in range(N_D):
                        nc.tensor.matmul(
                            pa[:],
                            w1_sb[i][:, d * 128:(d + 1) * 128],
                            tok_slice(ci, d),
                            start=(d == 0),
                            stop=(d == N_D - 1),
                        )
                    nc.scalar.activation(
                        hh[ci][:, i * TCH:(i + 1) * TCH],
                        pa[:],
                        RELU,
                        bias=w1_sb[i][:, H:H + 1],
                    )

            # Phase B: yT = (h.T @ W2).T, stationary = W2 blocks
            for hi in range(N_H):
                for ci in range(n_ch):
                    pb = ps.tile([128, TCH], F32, tag="p")
                    for i in range(N_I):
                        nc.tensor.matmul(
                            pb[:],
                            w2_sb[i][:, hi * 128:(hi + 1) * 128],
                            hh[ci][:, i * TCH:(i + 1) * TCH],
                            start=(i == 0),
                            stop=(i == N_I - 1),
                        )
                    if hi == N_H - 1 or n_ch % 2:
                        # final drain: a transfer walks ~47ns per partition
                        # regardless of width, so shrink the fragment count
                        # of the last group's stores (pieces walk in
                        # parallel)
                        oo = opool.tile([128, TCH], BF16, tag="o")
                        nc.scalar.copy(oo[:], pb[:])
                        pieces = 4 if ci == n_ch - 1 else 2
                        step = 128 // pieces
                        for h in range(pieces):
                            nc.sync.dma_start(
                                yt.ap()[hi, h * step:(h + 1) * step,
                                        ci * TCH:(ci + 1) * TCH],
                                oo[h * step:(h + 1) * step, :],
                            )
                    else:
                        if ci % 2 == 0:
                            ow = opool.tile([128, 2 * TCH], BF16, tag="ow")
                        nc.scalar.copy(ow[:, (ci % 2) * TCH:(ci % 2 + 1) * TCH], pb[:])
                        if ci % 2 == 1:
                            nc.sync.dma_start(
                                yt.ap()[hi, :, (ci - 1) * TCH:(ci + 1) * TCH], ow[:]
                            )

    nc.compile()
    return nc


def kernel(x, Wr, br, W1, b1, W2, b2):
    global LAST_RESULT
    x = np.asarray(x, dtype=np.float32)
    Wr = np.asarray(Wr, dtype=np.float32)
    br = np.asarray(br, dtype=np.float32)
    W1 = np.asarray(W1, dtype=np.float32)
    b1 = np.asarray(b1, dtype=np.float32)
    W2 = np.asarray(W2, dtype=np.float32)
    b2 = np.asarray(b2, dtype=np.float32)

    batch, seq, hidden = x.shape
    x2d = x.reshape(-1, hidden)
    n = x2d.shape[0]

    # Router (matches jax reference: top-2 descending, stable ties, softmax).
    logits = x2d @ Wr + br
    order = np.argsort(-logits, axis=1, kind="stable")[:, :TOP_K]
    l0 = logits[np.arange(n), order[:, 0]]
    l1 = logits[np.arange(n), order[:, 1]]
    e1 = np.exp(l1 - l0)
    denom = 1.0 + e1
    top_w = np.stack([1.0 / denom, e1 / denom], axis=1).astype(np.float32)

    rows_l, wsel_l = [], []
    for e in range(E):
        rows, cols = np.nonzero(order == e)
        rows_l.append(rows)
        wsel_l.append(top_w[rows, cols])
    counts = np.array([len(r) for r in rows_l])

    # Expert capacity: perfect-balance point (n*TOP_K/E). Overflow tokens
    # of hot experts are computed on the host in fp32 during the combine.
    cap = n * TOP_K // E
    c = max(TCH, min(int(-(-counts.max() // TCH)) * TCH, cap))

    if c not in _cache:
        _cache[c] = _build(c)
    nc = _cache[c]

    in_maps = []
    pad_ref = []
    for e in range(E):
        rows = rows_l[e][:c]
        ne = len(rows)
        xTe = np.zeros((H, c), dtype=NPBF16)
        xTe[:, :ne] = x2d[rows].T.astype(NPBF16)
        # pack to [ci, d, 128, TCH] so each DMA is a contiguous burst
        xte = np.ascontiguousarray(
            xTe.reshape(N_D, 128, c // TCH, TCH).transpose(2, 0, 1, 3)
        )
        w1t = np.empty((N_I, 128, H + 1), dtype=NPBF16)
        w1t[:, :, :H] = (
            W1[e].reshape(N_D, 128, N_I, 128).transpose(2, 1, 0, 3).reshape(N_I, 128, H)
        ).astype(NPBF16)
        w1t[:, :, H] = b1[e].reshape(N_I, 128).astype(NPBF16)
        in_maps.append(
            {
                "xt": xte,
                "w1t": w1t,
                "w2": W2[e].astype(NPBF16),
            }
        )
        # padded token columns all compute yT_pad = (relu(b1) @ W2).T
        pad_ref.append(np.maximum(b1[e], 0.0) @ W2[e])

    # Host fp32 reference for a few sampled real tokens per expert: the
    # device occasionally returns subtly corrupted data (~2e-2-level errors)
    # in the used region that the pad-column canary cannot see.
    spot_cols, spot_ref = [], []
    for e in range(E):
        ne = len(rows_l[e][:c])
        cols = (
            np.unique(np.linspace(0, ne - 1, 16).astype(int))
            if ne > 0
            else np.zeros(0, dtype=int)
        )
        xs = x2d[rows_l[e][cols]]
        hs = np.maximum(xs @ W1[e] + b1[e], 0.0)
        spot_cols.append(cols)
        spot_ref.append((hs @ W2[e]).T)  # [H, len(cols)]

    # The device occasionally drops a run (NRT_EXEC_UNIT_UNRECOVERABLE) and
    # the run after a drop can return garbage. Padded columns must come back
    # (a) bit-identical to each other and (b) close to the host-computed
    # relu(b1)@W2 — use that as an integrity canary and retry on failure.
    res = None
    for attempt in range(4):
        try:
            res = run_bass_kernel_spmd(
                nc, in_maps, list(range(E)), trace=TRACE, **TRACE_KWARGS
            )
        except Exception:
            if attempt == 3:
                raise
            continue
        ok = True
        for e in range(E):
            ye = _unpack_y(res.results[e]["yt"], c)
            ne = len(rows_l[e][:c])
            if not np.isfinite(ye).all():
                ok = False
                break
            if (
                spot_cols[e].size
                and np.abs(ye[:, spot_cols[e]] - spot_ref[e]).max() > 1e-2
            ):
                ok = False
                break
            if ne < c:
                v = pad_ref[e]
                tol = 0.05 * max(np.abs(v).max(), 1e-2)
                if (
                    np.abs(ye[:, ne:] - v[:, None]).max() > tol
                    or not (ye[:, ne:] == ye[:, -1:]).all()
                ):
                    ok = False
                    break
        if ok:
            break
    LAST_RESULT = res

    out = np.zeros((n, hidden), dtype=np.float32)
    for e in range(E):
        rows = rows_l[e][:c]
        ne = len(rows)
        w = wsel_l[e][:ne, None]
        ye = _unpack_y(res.results[e]["yt"], c)
        out[rows] += w * ye[:, :ne].T + w * b2[e][None, :]
        if len(rows_l[e]) > c:  # overflow tokens: full-precision host FFN
            rov = rows_l[e][c:]
            wov = wsel_l[e][c:, None]
            hov = np.maximum(x2d[rov] @ W1[e] + b1[e], 0.0)
            out[rov] += wov * (hov @ W2[e] + b2[e])
    return out.reshape(batch, seq, hidden)


# revision 48
# speedup vs baseline: 1.0186x; 1.0075x over previous
"""MoE layer (8 experts, top-2) on 8 TRN2 NeuronCores, expert-parallel.

Host does the router + dispatch/combine; each core runs the two FFN matmuls
for one expert on its gathered tokens in bf16 (fp32 PSUM accumulation).

Two serial phases, both with static weight tiles as the matmul stationary
operand so the PE weight loads hide under the previous matmul's stream
(stationary tiles freshly written by another engine stall the load on the
producer semaphore — measured +40ns/matmul on the fp32r baseline):
  A: h[i-slab, tok] = relu(W1-block.T @ x-chunk + b1), 512 matmuls
  B: yT[h-block, tok] = W2-block.T @ h-chunk,          512 matmuls
Output is the transposed yT = (relu(x@W1+b1) @ W2).T; the host transposes,
scales by the router weight and adds w*b2 during the combine.

Self-contained: hardcodes shapes HIDDEN=1024, INNER=2048, NUM_EXPERTS=8,
TOP_K=2.
"""

import sys

import numpy as np
import ml_dtypes

try:
    import concourse.bass as bass  # noqa: F401
except ImportError:
    sys.path.insert(0, "/opt/trn_rl_repo")

import concourse.tile as tile
from concourse import bacc, mybir
from concourse.bass_utils import run_bass_kernel_spmd

H = 1024
INNER = 2048
E = 8
TOP_K = 2
N_D = H // 128  # 8 k-tiles for matmul A
N_I = INNER // 128  # 16 k-tiles for matmul B
N_H = H // 128  # 8 output row-blocks
TCH = 512  # token chunk (moving free dim, PE max)

F32 = mybir.dt.float32
BF16 = mybir.dt.bfloat16
NPBF16 = ml_dtypes.bfloat16
RELU = mybir.ActivationFunctionType.Relu

# test.py hooks: set TRACE=True before calling kernel() to profile;
# LAST_RESULT then holds the BassKernelResults (exec_time_ns etc.).
TRACE = False
TRACE_KWARGS = {}
LAST_RESULT = None

_cache = {}


def _unpack_y(yt, c):
    # yt [N_H, 128, c] -> yT [H, c] float32
    return np.asarray(yt, dtype=np.float32).reshape(H, c)


def _build(c):
    n_ch = c // TCH
    nc = bacc.Bacc("TRN2", target_bir_lowering=False, debug=False, num_devices=8)

    # All DRAM tensors are packed on the host so every DMA moves a
    # contiguous region — the DMA engine fragments strided transfers into
    # per-line descriptors at ~47ns/line, which throttles 1KB-line tiles
    # to ~21GB/s/queue and turns the output drain into a ~10us tail.
    # xt[ci][d] = xT[d*128:(d+1)*128, ci*512:(ci+1)*512]
    xt = nc.dram_tensor("xt", [n_ch, N_D, 128, TCH], BF16, kind="ExternalInput")
    # W1 pre-tiled on host into inner-dim slabs: w1t[i][p, d*128+m] =
    # W1[d*128+p, i*128+m], so slab i holds all stationary blocks for
    # phase-A step i. Column H carries the slab's bias b1[i*128+p] (bf16
    # bias costs ~1e-4 absolute — noise) so no separate b1 DMA: even a
    # 64B/partition transfer walks one ~47ns fragment per partition, i.e.
    # ~6us of a DMA walker slot in the critical warmup window.
    w1 = nc.dram_tensor("w1t", [N_I, 128, H + 1], BF16, kind="ExternalInput")
    w2 = nc.dram_tensor("w2", [INNER, H], BF16, kind="ExternalInput")
    # yt[hi][p][tok] = yT[hi*128+p, tok] — i.e. plain yT [H, c] rows.
    # Stores go out as [128, 2*TCH] chunk-pairs: 2KB contiguous per
    # partition row, so each walk moves twice the bytes of a single-chunk
    # store for the same ~47ns/partition fragment cost.
    yt = nc.dram_tensor("yt", [N_H, 128, c], BF16, kind="ExternalOutput")

    with tile.TileContext(nc, pool_alloc_mode="queue") as tc:
        with (
            tc.tile_pool(name="weights", bufs=1) as wpool,
            tc.tile_pool(name="tokens", bufs=1) as tpool,
            tc.tile_pool(name="hidden", bufs=1) as hpool,
            tc.tile_pool(name="out", bufs=4) as opool,
            tc.tile_pool(name="psum", bufs=8, space="PSUM") as ps,
        ):
            warm = wpool.tile([128, 640], BF16, tag="warm")
            w1_sb = [
                wpool.tile([128, H + 1], BF16, tag=f"w1_{i}", name=f"w1s_{i}")
                for i in range(N_I)
            ]
            w2_sb = [
                wpool.tile([128, H], BF16, tag=f"w2_{i}", name=f"w2s_{i}")
                for i in range(N_I)
            ]
            # token chunk c as two tiles (d=0..3 / d=4..7) so the first
            # matmul only gates on half the chunk's DMA
            tt = [
                (
                    tpool.tile([128, 4 * TCH], BF16, tag=f"tl_{ci}", name=f"tl_{ci}"),
                    tpool.tile([128, 4 * TCH], BF16, tag=f"th_{ci}", name=f"th_{ci}"),
                )
                for ci in range(n_ch)
            ]
            hh = [
                hpool.tile([128, N_I * TCH], BF16, tag=f"h_{ci}", name=f"h_{ci}")
                for ci in range(n_ch)
            ]

            def tok_slice(ci, d):
                lo, hi = tt[ci]
                t = lo if d < 4 else hi
                return t[:, (d % 4) * TCH:(d % 4 + 1) * TCH]

            # PE pre-warm: the pstate governor needs ~3us of continuous
            # execution before the PE runs at 2.4GHz (cold matmuls take
            # 427-609ns instead of 216ns). Bridge the initial DMA wait with
            # dummy matmuls on a memset tile (vector engine, no DMA dep),
            # aimed at the first real psum tile — its real group opens with
            # start=True, which resets the bank, so the junk is discarded
            # and the tile keeps its reader.
            nc.vector.memset(warm[:], 1.0)
            pa0 = ps.tile([128, TCH], F32, tag="p")
            for _ in range(10):
                nc.tensor.matmul(
                    pa0[:], warm[:, 0:128], warm[:, 128:640], start=True, stop=True
                )

            # DMA order = strict consumption order. Descriptor issue costs
            # ~0.3us each on the sync sequencer, so keep the count low: one
            # contiguous 128-256KB burst per tile region. Phase A consumes
            # a w1 slab every ~1.7us; token chunks are needed at 27/55/82us
            # and w2 only at B start (~118us), so w2 goes dead last to keep
            # it out of the contended warmup window.
            nc.sync.dma_start(w1_sb[0][:], w1.ap()[0])
            for d in range(N_D):
                nc.sync.dma_start(tok_slice(0, d), xt.ap()[0, d])
            nc.sync.dma_start(w1_sb[1][:], w1.ap()[1])
            nc.sync.dma_start(w1_sb[2][:], w1.ap()[2])
            for i in range(3, N_I):
                nc.sync.dma_start(w1_sb[i][:], w1.ap()[i])
            for ci in range(1, n_ch):
                for d in range(N_D):
                    nc.sync.dma_start(tok_slice(ci, d), xt.ap()[ci, d])
            for i in range(N_I):
                nc.sync.dma_start(w2_sb[i][:], w2.ap()[i * 128:(i + 1) * 128, :])

            # Phase A: h = relu(x @ W1 + b1), h laid out [inner-part, tok]
            for ci in range(n_ch):
                for i in range(N_I):
                    pa = pa0 if (ci == 0 and i == 0) else ps.tile([128, TCH], F32, tag="p")
                    for d in range(N_D):
                        nc.tensor.matmul(
                            pa[:],
                            w1_sb[i][:, d * 128:(d + 1) * 128],
                            tok_slice(ci, d),
                            start=(d == 0),
                            stop=(d == N_D - 1),
                        )
                    nc.scalar.activation(
                        hh[ci][:, i * TCH:(i + 1) * TCH],
                        pa[:],
                        RELU,
                        bias=w1_sb[i][:, H:H + 1],
                    )

            # Phase B: yT = (h.T @ W2).T, stationary = W2 blocks
            for hi in range(N_H):
                for ci in range(n_ch):
                    pb = ps.tile([128, TCH], F32, tag="p")
                    for i in range(N_I):
                        nc.tensor.matmul(
                            pb[:],
                            w2_sb[i][:, hi * 128:(hi + 1) * 128],
                            hh[ci][:, i * TCH:(i + 1) * TCH],
                            start=(i == 0),
                            stop=(i == N_I - 1),
                        )
                    if hi == N_H - 1 or n_ch % 2:
                        # final drain: a transfer walks ~47ns per partition
                        # regardless of width, so shrink the fragment count
                        # of the last group's stores (pieces walk in
                        # parallel)
                        oo = opool.tile([128, TCH], BF16, tag="o")
                        if ci == n_ch - 1:
                            # very last tile: pipeline copy/issue/walk —
                            # copy partition-halves separately and issue
                            # the store pieces from two sequencers so the
                            # first pieces walk while the second half is
                            # still copying
                            for g in range(2):
                                sl = slice(g * 64, (g + 1) * 64)
                                nc.scalar.copy(oo[sl, :], pb[sl, :])
                                eng = nc.sync if g == 0 else nc.scalar
                                for h in range(2):
                                    p0 = g * 64 + h * 32
                                    eng.dma_start(
                                        yt.ap()[hi, p0:p0 + 32,
                                                ci * TCH:(ci + 1) * TCH],
                                        oo[p0:p0 + 32, :],
                                    )
                        else:
                            nc.scalar.copy(oo[:], pb[:])
                            for h in range(2):
                                nc.sync.dma_start(
                                    yt.ap()[hi, h * 64:(h + 1) * 64,
                                            ci * TCH:(ci + 1) * TCH],
                                    oo[h * 64:(h + 1) * 64, :],
                                )
                    else:
                        if ci % 2 == 0:
                            ow = opool.tile([128, 2 * TCH], BF16, tag="ow")
                        nc.scalar.copy(ow[:, (ci % 2) * TCH:(ci % 2 + 1) * TCH], pb[:])
                        if ci % 2 == 1:
                            nc.sync.dma_start(
                                yt.ap()[hi, :, (ci - 1) * TCH:(ci + 1) * TCH], ow[:]
                            )

    nc.compile()
    return nc


def kernel(x, Wr, br, W1, b1, W2, b2):
    global LAST_RESULT
    x = np.asarray(x, dtype=np.float32)
    Wr = np.asarray(Wr, dtype=np.float32)
    br = np.asarray(br, dtype=np.float32)
    W1 = np.asarray(W1, dtype=np.float32)
    b1 = np.asarray(b1, dtype=np.float32)
    W2 = np.asarray(W2, dtype=np.float32)
    b2 = np.asarray(b2, dtype=np.float32)

    batch, seq, hidden = x.shape
    x2d = x.reshape(-1, hidden)
    n = x2d.shape[0]

    # Router (matches jax reference: top-2 descending, stable ties, softmax).
    logits = x2d @ Wr + br
    order = np.argsort(-logits, axis=1, kind="stable")[:, :TOP_K]
    l0 = logits[np.arange(n), order[:, 0]]
    l1 = logits[np.arange(n), order[:, 1]]
    e1 = np.exp(l1 - l0)
    denom = 1.0 + e1
    top_w = np.stack([1.0 / denom, e1 / denom], axis=1).astype(np.float32)

    rows_l, wsel_l = [], []
    for e in range(E):
        rows, cols = np.nonzero(order == e)
        rows_l.append(rows)
        wsel_l.append(top_w[rows, cols])
    counts = np.array([len(r) for r in rows_l])

    # Expert capacity: perfect-balance point (n*TOP_K/E). Overflow tokens
    # of hot experts are computed on the host in fp32 during the combine.
    cap = n * TOP_K // E
    c = max(TCH, min(int(-(-counts.max() // TCH)) * TCH, cap))

    if c not in _cache:
        _cache[c] = _build(c)
    nc = _cache[c]

    in_maps = []
    pad_ref = []
    for e in range(E):
        rows = rows_l[e][:c]
        ne = len(rows)
        xTe = np.zeros((H, c), dtype=NPBF16)
        xTe[:, :ne] = x2d[rows].T.astype(NPBF16)
        # pack to [ci, d, 128, TCH] so each DMA is a contiguous burst
        xte = np.ascontiguousarray(
            xTe.reshape(N_D, 128, c // TCH, TCH).transpose(2, 0, 1, 3)
        )
        w1t = np.empty((N_I, 128, H + 1), dtype=NPBF16)
        w1t[:, :, :H] = (
            W1[e].reshape(N_D, 128, N_I, 128).transpose(2, 1, 0, 3).reshape(N_I, 128, H)
        ).astype(NPBF16)
        w1t[:, :, H] = b1[e].reshape(N_I, 128).astype(NPBF16)
        in_maps.append(
            {
                "xt": xte,
                "w1t": w1t,
                "w2": W2[e].astype(NPBF16),
            }
        )
        # padded token columns all compute yT_pad = (relu(b1) @ W2).T
        pad_ref.append(np.maximum(b1[e], 0.0) @ W2[e])

    # Host fp32 reference for a few sampled real tokens per expert: the
    # device occasionally returns subtly corrupted data (~2e-2-level errors)
    # in the used region that the pad-column canary cannot see.
    spot_cols, spot_ref = [], []
    for e in range(E):
        ne = len(rows_l[e][:c])
        cols = (
            np.unique(np.linspace(0, ne - 1, 16).astype(int))
            if ne > 0
            else np.zeros(0, dtype=int)
        )
        xs = x2d[rows_l[e][cols]]
        hs = np.maximum(xs @ W1[e] + b1[e], 0.0)
        spot_cols.append(cols)
        spot_ref.append((hs @ W2[e]).T)  # [H, len(cols)]

    # The device occasionally drops a run (NRT_EXEC_UNIT_UNRECOVERABLE) and
    # the run after a drop can return garbage. Padded columns must come back
    # (a) bit-identical to each other and (b) close to the host-computed
    # relu(b1)@W2 — use that as an integrity canary and retry on failure.
    res = None
    for attempt in range(4):
        try:
            res = run_bass_kernel_spmd(
                nc, in_maps, list(range(E)), trace=TRACE, **TRACE_KWARGS
            )
        except Exception:
            if attempt == 3:
                raise
            continue
        ok = True
        for e in range(E):
            ye = _unpack_y(res.results[e]["yt"], c)
            ne = len(rows_l[e][:c])
            if not np.isfinite(ye).all():
                ok = False
                break
            if (
                spot_cols[e].size
                and np.abs(ye[:, spot_cols[e]] - spot_ref[e]).max() > 1e-2
            ):
                ok = False
                break
            if ne < c:
                v = pad_ref[e]
                tol = 0.05 * max(np.abs(v).max(), 1e-2)
                if (
                    np.abs(ye[:, ne:] - v[:, None]).max() > tol
                    or not (ye[:, ne:] == ye[:, -1:]).all()
                ):
                    ok = False
                    break
        if ok:
            break
    LAST_RESULT = res

    out = np.zeros((n, hidden), dtype=np.float32)
    for e in range(E):
        rows = rows_l[e][:c]
        ne = len(rows)
        w = wsel_l[e][:ne, None]
        ye = _unpack_y(res.results[e]["yt"], c)
        out[rows] += w * ye[:, :ne].T + w * b2[e][None, :]
        if len(rows_l[e]) > c:  # overflow tokens: full-precision host FFN
            rov = rows_l[e][c:]
            wov = wsel_l[e][c:, None]
            hov = np.maximum(x2d[rov] @ W1[e] + b1[e], 0.0)
            out[rov] += wov * (hov @ W2[e] + b2[e])
    return out.reshape(batch, seq, hidden)
